# revision 1
# baseline (speedup 1.0000x reference)
"""CAVLossV1 on 8 Trainium2 NeuronCores (Bass/Tile), data-parallel over batch.

Full (unsharded) inputs in, full scalar loss out. Internally:
  - batch B=512 split 64/core across 8 cores
  - each core computes partial sums (focal-loss terms, alignment pos/neg
    sums+counts, text-anchor sums); host combines the 8 partial vectors.

Self-contained: only imports infra from /opt/trn_rl_repo.
"""
import sys

sys.path.insert(0, "/opt/trn_rl_repo")

import numpy as np
from contextlib import ExitStack

import concourse.bass as bass
import concourse.tile as tile
from concourse import mybir
from concourse.vector_clock import ScopedClock

F32 = mybir.dt.float32
F32R = mybir.dt.float32r
BF16 = mybir.dt.bfloat16
U32 = mybir.dt.uint32
AF = mybir.ActivationFunctionType
ALU = mybir.AluOpType
AX = mybir.AxisListType

N_CORES = 8
B, P, C, D, K = 512, 196, 80, 512, 16
BSH = B // N_CORES            # 64 samples per core
NCH = BSH * K // 128          # 8 chunks of 128 (sample,k) rows
SPC = 128 // K                # 8 samples per chunk
PL, PH = 128, P - 128         # upper/lower patch split (128 + 68)
ALPHA, BETA, TAU = 0.3, 1.0, 0.07
CLASS_W = np.array([2.5, 2.0, 1.0, 5.0, 4.0, 5.0, 5.0], dtype=np.float32)


def _conf_mat():
    m = np.ones((7, 7), dtype=np.float32)
    pairs = {(0, 1): 3.0, (1, 0): 3.0, (0, 2): 2.0, (2, 0): 2.0, (1, 2): 2.0,
             (2, 1): 2.0, (5, 0): 2.0, (0, 5): 2.0, (4, 6): 2.0, (6, 4): 2.0}
    for (i, j), w in pairs.items():
        m[i, j] = w
    return m


CONF_MAT = _conf_mat()

# ---------------------------------------------------------------------------
# Tile patch: this container's walrus accepts only ONE sync-wait per TPB
# instruction; Tile attaches all needed waits to the consuming instruction.
# Split any multi-wait instruction into single-wait nop carriers.
# ---------------------------------------------------------------------------
_orig_lower = tile.TileContext._lower_ordered_insts


def _split_waits(self, ordered):
    nc = self.nc
    for insts in ordered.values():
        i = 0
        while i < len(insts):
            inst = insts[i]
            si = inst.sync_info
            if si is not None and len(si.on_wait) > 1:
                waits = list(si.on_wait)
                carriers = []
                for w in waits[:-1]:
                    nop = mybir.InstNoOp(name=f"waitnop-{nc.next_id()}", ins=[], outs=[])
                    nop.engine = inst.engine
                    nop.sync_info = mybir.SyncInfo(on_wait=[w], on_update=[])
                    carriers.append(nop)
                inst.sync_info = mybir.SyncInfo(on_wait=[waits[-1]], on_update=list(si.on_update))
                insts[i:i] = carriers
                i += len(carriers)
            i += 1


def _patched_lower(self, ordered):
    _split_waits(self, ordered)
    return _orig_lower(self, ordered)


def _patched_drain_and_barrier(self, tick_clock, wait_clock):
    nc = self.nc
    carrier = nc.sync.nop()
    wait_clock.add_sem_waits(carrier.ins, ScopedClock({None: tick_clock.global_clock}))
    si = carrier.ins.sync_info
    if si is not None and len(si.on_wait) > 1:
        carrier.ins.sync_info = mybir.SyncInfo(on_wait=[si.on_wait[0]], on_update=list(si.on_update))
        for w in si.on_wait[1:]:
            extra = nc.sync.nop()
            extra.ins.sync_info = mybir.SyncInfo(on_wait=[w], on_update=[])
    nc.sync.drain()
    nc.all_engine_barrier()
    assert self.sems is not None
    popped = nc._tile_sem_poison_stack.pop()
    assert popped is self._sem_poison
    nc.clear_and_free_semaphores(list(self.sems.allocated().values()))
    nc.all_engine_barrier()


tile.TileContext._drain_and_barrier = _patched_drain_and_barrier
tile.TileContext._lower_ordered_insts = _patched_lower


# ---------------------------------------------------------------------------
# Kernel build
# ---------------------------------------------------------------------------
def _build_module():
    nc = bass.Bass("TRN2", debug=False)

    def din(name, shape, dt=F32):
        return nc.dram_tensor(name, list(shape), dt, kind="ExternalInput").ap()

    dl_d = din("dl", [BSH, 7])
    cs_d = din("cs", [BSH, C])
    ps_d = din("ps", [BSH, P, C])
    pf_d = din("pf", [BSH, P, D])
    cav_d = din("cav", [C, D])
    wv_d = din("wv", [7, C])
    te_d = din("te", [C, D])
    cnt_d = din("cnt", [C, BSH])
    iota_d = din("iota80", [128, C])
    i128_d = din("ident", [128, 128])
    i80_d = din("ident80", [C, C])
    ohlab_d = din("ohlab", [BSH, 7])
    confrow_d = din("confrow", [BSH, 7])
    clsw_d = din("clsw", [BSH, 1])
    labt_d = din("labt", [7, BSH * K])

    idx_scr = nc.dram_tensor("idx_scr", [BSH * K], U32).ap()
    out_d = nc.dram_tensor("out", [1, 8], F32, kind="ExternalOutput").ap()

    with tile.TileContext(nc) as tc, ExitStack() as ctx:
        cpool = ctx.enter_context(tc.tile_pool(name="consts", bufs=1))
        spool = ctx.enter_context(tc.tile_pool(name="setup", bufs=1))
        wpool = ctx.enter_context(tc.tile_pool(name="work", bufs=2))
        # PSUM budget (8 banks): tp=3, big=2, mpg=2  -> 7 banks
        ps_tp = ctx.enter_context(tc.tile_pool(name="ps_tp", bufs=3, space="PSUM"))
        ps_big = ctx.enter_context(tc.tile_pool(name="ps_big", bufs=2, space="PSUM"))
        ps_mpg = ctx.enter_context(tc.tile_pool(name="ps_mpg", bufs=2, space="PSUM"))

        def load(pool, ap_, shape, dt=F32, tag=None):
            t = pool.tile(shape, dt, tag=tag)
            nc.sync.dma_start(t[:], ap_[:])
            return t

        IOTA = load(cpool, iota_d, [128, C], tag="IOTA")
        I128 = load(cpool, i128_d, [128, 128], tag="I128")
        I80 = load(cpool, i80_d, [C, C], tag="I80")
        OHLAB = load(cpool, ohlab_d, [BSH, 7], tag="OHLAB")
        CONFROW = load(cpool, confrow_d, [BSH, 7], tag="CONFROW")
        CLSW = load(cpool, clsw_d, [BSH, 1], tag="CLSW")
        LABT = load(cpool, labt_d, [7, BSH * K], tag="LABT")
        CNT = load(cpool, cnt_d, [C, BSH], tag="CNT")
        dl = load(spool, dl_d, [BSH, 7], tag="dl")
        cs = load(spool, cs_d, [BSH, C], tag="cs")
        cav = load(spool, cav_d, [C, D], tag="cav")
        wv = load(spool, wv_d, [7, C], tag="wv")
        te = load(spool, te_d, [C, D], tag="te")

        # ---- accumulators ----
        ACCF = cpool.tile([128, 8], F32, tag="ACCF")
        nc.gpsimd.memset(ACCF[:], 0.0)
        NUM = cpool.tile([128, NCH], F32, tag="NUM")
        SSQMP = cpool.tile([128, NCH], F32, tag="SSQMP")
        DW = cpool.tile([128, NCH], F32, tag="DWA")

        # =============== row-normalizations ===============
        def l2rows(t, n, d, tag, eps_add=None, eps_max=None):
            scr = spool.tile([n, d], F32, tag="normscr")
            ss = spool.tile([n, 1], F32, tag="normss")
            nc.vector.scalar_tensor_tensor(scr[:], t[:], 1.0, t[:], ALU.mult, ALU.mult, accum_out=ss[:])
            nrm = spool.tile([n, 1], F32, tag="normv")
            nc.scalar.sqrt(nrm[:], ss[:])
            if eps_add is not None:
                nc.vector.tensor_scalar(nrm[:], nrm[:], float(eps_add), None, op0=ALU.add)
            if eps_max is not None:
                nc.vector.tensor_scalar(nrm[:], nrm[:], float(eps_max), None, op0=ALU.max)
            inv = spool.tile([n, 1], F32, tag="normi")
            nc.vector.reciprocal(inv[:], nrm[:])
            out = spool.tile([n, d], F32, tag=tag)
            nc.vector.tensor_scalar(out[:], t[:], inv[:], None, op0=ALU.mult)
            return out

        cav_n = l2rows(cav, C, D, "cav_n", eps_add=1e-8)
        te_n = l2rows(te, C, D, "te_n", eps_max=1e-12)
        wv_n = l2rows(wv, 7, C, "wv_n", eps_max=1e-12)
        cav_nr = spool.tile([C, D], F32R, tag="cav_nr")
        nc.vector.tensor_copy(cav_nr[:], cav_n[:])

        # =============== text anchor loss ===============
        cavT = spool.tile([128, 4 * C], F32, tag="cavT")
        teT = spool.tile([128, 4 * C], F32, tag="teT")
        for j in range(4):
            pt = ps_tp.tile([128, 128], F32, tag="tp")
            nc.tensor.transpose(pt[0:128, 0:C], cav_n[:, 128 * j:128 * (j + 1)], I128[0:C, 0:C])
            nc.scalar.copy(cavT[:, C * j:C * (j + 1)], pt[0:128, 0:C])
            pt2 = ps_tp.tile([128, 128], F32, tag="tp")
            nc.tensor.transpose(pt2[0:128, 0:C], te_n[:, 128 * j:128 * (j + 1)], I128[0:C, 0:C])
            nc.scalar.copy(teT[:, C * j:C * (j + 1)], pt2[0:128, 0:C])
        # GT[c', c] = sum_d te_n[c', d] cav_n[c, d]  (= G^T; diag(GT)=diag(G))
        GT = ps_mpg.tile([C, C], F32, tag="mpg")
        for j in range(4):
            nc.tensor.matmul(GT[:], teT[:, C * j:C * (j + 1)], cavT[:, C * j:C * (j + 1)],
                             start=(j == 0), stop=(j == 3))
        ET = spool.tile([C, C], F32, tag="ET")
        nc.scalar.activation(ET[:], GT[:], AF.Exp, scale=1.0 / TAU)
        diagG = spool.tile([C, 1], F32, tag="diagG")
        scr80 = spool.tile([C, C], F32, tag="scr80")
        nc.vector.scalar_tensor_tensor(scr80[:], GT[:], 1.0, I80[:], ALU.mult, ALU.mult, accum_out=diagG[:])
        V = ps_mpg.tile([C, BSH], F32, tag="mpg")
        nc.tensor.matmul(V[:], ET[:], CNT[:], start=True, stop=True)
        LNV = spool.tile([C, BSH], F32, tag="LNV")
        nc.scalar.activation(LNV[:], V[:], AF.Ln)
        scr64 = spool.tile([C, BSH], F32, tag="scr64")
        nc.vector.scalar_tensor_tensor(scr64[:], LNV[:], 1.0, CNT[:], ALU.mult, ALU.mult,
                                       accum_out=ACCF[0:C, 4:5])
        rowcnt = spool.tile([C, 1], F32, tag="rowcnt")
        nc.vector.tensor_reduce(rowcnt[:], CNT[:], axis=AX.X, op=ALU.add)
        dterm = spool.tile([C, 1], F32, tag="dterm")
        nc.vector.tensor_tensor(dterm[:], diagG[:], rowcnt[:], op=ALU.mult)
        nc.vector.tensor_scalar(ACCF[0:C, 5:6], dterm[:], 1.0 / TAU, None, op0=ALU.mult)

        # =============== focal loss ===============
        m = spool.tile([BSH, 1], F32, tag="fm")
        nc.vector.tensor_reduce(m[:], dl[:], axis=AX.X, op=ALU.max)
        mneg = spool.tile([BSH, 1], F32, tag="fmneg")
        nc.vector.tensor_scalar(mneg[:], m[:], -1.0, None, op0=ALU.mult)
        ex = spool.tile([BSH, 7], F32, tag="fex")
        sumexp = spool.tile([BSH, 1], F32, tag="fse")
        nc.scalar.activation(ex[:], dl[:], AF.Exp, bias=mneg[:], accum_out=sumexp[:])
        lse = spool.tile([BSH, 1], F32, tag="flse")
        nc.scalar.activation(lse[:], sumexp[:], AF.Ln)
        xl = spool.tile([BSH, 1], F32, tag="fxl")
        scr7 = spool.tile([BSH, 7], F32, tag="fscr7")
        nc.vector.scalar_tensor_tensor(scr7[:], dl[:], 1.0, OHLAB[:], ALU.mult, ALU.mult, accum_out=xl[:])
        ce = spool.tile([BSH, 1], F32, tag="fce")
        nc.vector.tensor_tensor(ce[:], m[:], lse[:], op=ALU.add)
        nc.vector.tensor_tensor(ce[:], ce[:], xl[:], op=ALU.subtract)
        pt_ = spool.tile([BSH, 1], F32, tag="fpt")
        nc.scalar.activation(pt_[:], ce[:], AF.Exp, scale=-1.0)
        omp = spool.tile([BSH, 1], F32, tag="fomp")
        nc.vector.tensor_scalar(omp[:], pt_[:], -1.0, 1.0, op0=ALU.mult, op1=ALU.add)
        nc.vector.tensor_scalar(omp[:], omp[:], 1e-30, None, op0=ALU.max)
        lnomp = spool.tile([BSH, 1], F32, tag="flnomp")
        nc.scalar.activation(lnomp[:], omp[:], AF.Ln)
        focal = spool.tile([BSH, 1], F32, tag="ffw")
        nc.scalar.activation(focal[:], lnomp[:], AF.Exp, scale=2.5)
        cp = spool.tile([BSH, 1], F32, tag="fcp")
        nc.vector.tensor_scalar(cp[:], sumexp[:], float(1.0 / 0.7), None, op0=ALU.is_gt)
        nc.vector.tensor_scalar(cp[:], cp[:], 0.5, 1.0, op0=ALU.mult, op1=ALU.add)
        eqm = spool.tile([BSH, 7], F32, tag="feqm")
        nc.vector.tensor_scalar(eqm[:], dl[:], m[:], None, op0=ALU.is_equal)
        cw = spool.tile([BSH, 1], F32, tag="fcw")
        nc.vector.scalar_tensor_tensor(scr7[:], eqm[:], 1.0, CONFROW[:], ALU.mult, ALU.mult, accum_out=cw[:])
        wce = spool.tile([BSH, 1], F32, tag="fwce")
        nc.vector.tensor_tensor(wce[:], focal[:], ce[:], op=ALU.mult)
        nc.vector.tensor_tensor(wce[:], wce[:], CLSW[:], op=ALU.mult)
        nc.vector.tensor_tensor(wce[:], wce[:], cw[:], op=ALU.mult)
        nc.vector.tensor_tensor(wce[:], wce[:], cp[:], op=ALU.mult)
        nc.vector.tensor_copy(ACCF[0:BSH, 0:1], wce[:])

        # =============== concept top-16 ===============
        m8a = spool.tile([BSH, 8], F32, tag="m8a")
        nc.vector.max(m8a[:], cs[:])
        i8a = spool.tile([BSH, 8], U32, tag="i8a")
        nc.vector.max_index(i8a[:], m8a[:], cs[:])
        csk = spool.tile([BSH, C], F32, tag="csk")
        nc.vector.match_replace(csk[:], m8a[:], cs[:], -1e30)
        m8b = spool.tile([BSH, 8], F32, tag="m8b")
        nc.vector.max(m8b[:], csk[:])
        i8b = spool.tile([BSH, 8], U32, tag="i8b")
        nc.vector.max_index(i8b[:], m8b[:], csk[:])
        idx16 = spool.tile([BSH, K], U32, tag="idx16")
        nc.vector.tensor_copy(idx16[:, 0:8], i8a[:])
        nc.vector.tensor_copy(idx16[:, 8:16], i8b[:])
        nc.sync.dma_start(idx_scr[:].rearrange("(a b) -> a b", a=BSH), idx16[:])
        idxcols_u = spool.tile([128, NCH], U32, tag="idxcu")
        nc.sync.dma_start(idxcols_u[:], idx_scr[:].rearrange("(c p) -> p c", p=128))
        idxcols = spool.tile([128, NCH], F32, tag="idxcf")
        nc.vector.tensor_copy(idxcols[:], idxcols_u[:])

        wz_tiles = []
        for par in range(2):
            wzu_p = cpool.tile([PL, SPC * 160], BF16, tag=f"wzuP{par}")
            nc.gpsimd.memset(wzu_p[:], 0.0)
            wzl_p = cpool.tile([PH, SPC * 160], BF16, tag=f"wzlP{par}")
            nc.gpsimd.memset(wzl_p[:], 0.0)
            wz_tiles.append((wzu_p, wzl_p))

        # =============== chunk loop ===============
        for ci in range(NCH):
            s0 = ci * SPC
            # --- issue all chunk DMAs first (ps + pf) ---
            psu8 = wpool.tile([PL, SPC * C], F32, tag="psu8", bufs=3)
            nc.sync.dma_start(psu8[:].rearrange("p (a b) -> p a b", a=SPC),
                              ps_d[s0:s0 + SPC, 0:PL, :].transpose([1, 0, 2]))
            psl8 = wpool.tile([PH, SPC * C], F32, tag="psl8", bufs=3)
            nc.sync.dma_start(psl8[:].rearrange("p (a b) -> p a b", a=SPC),
                              ps_d[s0:s0 + SPC, PL:P, :].transpose([1, 0, 2]))
            pfu8 = wpool.tile([PL, SPC * D], F32, tag="pfu8", bufs=2)
            nc.sync.dma_start(pfu8[:].rearrange("p (a b) -> p a b", a=SPC),
                              pf_d[s0:s0 + SPC, 0:PL, :].transpose([1, 0, 2]))
            pfl8 = wpool.tile([PH, SPC * D], F32, tag="pfl8", bufs=2)
            nc.sync.dma_start(pfl8[:].rearrange("p (a b) -> p a b", a=SPC),
                              pf_d[s0:s0 + SPC, PL:P, :].transpose([1, 0, 2]))

            # --- onehot of selected concepts, chunk layout [128(bk), 80] ---
            oh = wpool.tile([128, C], F32, tag="oh")
            nc.vector.tensor_scalar(oh[:], IOTA[:], idxcols[:, ci:ci + 1], None, op0=ALU.is_equal)
            ohT_ps = ps_tp.tile([C, 128], F32, tag="tp")
            nc.tensor.transpose(ohT_ps[:], oh[:], I128[:])
            ohT = wpool.tile([C, 128], F32R, tag="ohTs")
            nc.vector.tensor_copy(ohT[:], ohT_ps[:])

            # --- ps transpose -> psT ---
            psT = wpool.tile([C, SPC * P], F32, tag="psT", bufs=3)
            for j in range(SPC):
                ptu = ps_tp.tile([C, PL], F32, tag="tp")
                nc.tensor.transpose(ptu[:], psu8[:, C * j:C * (j + 1)], I128[:])
                ptl = ps_tp.tile([C, PH], F32, tag="tp")
                nc.tensor.transpose(ptl[:], psl8[:, C * j:C * (j + 1)], I128[0:PH, 0:PH])
                if j % 2 == 0:
                    nc.vector.tensor_copy(psT[:, P * j:P * j + PL], ptu[:])
                    nc.vector.tensor_copy(psT[:, P * j + PL:P * (j + 1)], ptl[:])
                else:
                    nc.scalar.copy(psT[:, P * j:P * j + PL], ptu[:])
                    nc.scalar.copy(psT[:, P * j + PL:P * (j + 1)], ptl[:])
            ohT32 = wpool.tile([C, 128], F32, tag="ohT32")
            nc.vector.tensor_copy(ohT32[:], ohT_ps[:])

            # --- gather selected concept rows: [128, 1568] in 4 matmuls ---
            gat = wpool.tile([128, SPC * P], F32, tag="gat", bufs=2)
            NQ = SPC * P // 4  # 392
            for q in range(4):
                gps = ps_big.tile([128, NQ], F32, tag="big")
                nc.tensor.matmul(gps[:], ohT32[:], psT[:, NQ * q:NQ * (q + 1)], start=True, stop=True)
                nc.scalar.copy(gat[:, NQ * q:NQ * (q + 1)], gps[:])

            # --- diag-extract to per-row ps_sel [128, 196] ---
            ps_sel = wpool.tile([128, P], F32, tag="ps_sel", bufs=3)
            for j in range(SPC):
                nc.gpsimd.dma_start(ps_sel[K * j:K * (j + 1), :], gat[K * j:K * (j + 1), P * j:P * (j + 1)])

            # --- top-16 patches -> mask ---
            q8 = wpool.tile([128, 8], F32, tag="q8")
            nc.vector.max(q8[:], ps_sel[:])
            psk = wpool.tile([128, P], F32, tag="psk")
            nc.vector.match_replace(psk[:], q8[:], ps_sel[:], -1e30)
            q8b = wpool.tile([128, 8], F32, tag="q8b")
            nc.vector.max(q8b[:], psk[:])
            mask = wpool.tile([128, P], F32, tag="mask")
            nc.vector.tensor_scalar(mask[:], ps_sel[:], q8b[:, 7:8], None, op0=ALU.is_ge)

            # --- pf load (batched), bf16 convert (Pool), ssq (ACT/DVE split) ---
            SSQU = wpool.tile([PL, SPC], F32, tag="ssqu")
            SSQL = wpool.tile([PH, SPC], F32, tag="ssql")
            pfb_u, pfb_l = [], []
            for j in range(SPC):
                pfu = pfu8[:, D * j:D * (j + 1)]
                pfl = pfl8[:, D * j:D * (j + 1)]
                bu = wpool.tile([PL, D], BF16, tag=f"bu{j}")
                nc.gpsimd.tensor_copy(bu[:], pfu)
                bl = wpool.tile([PH, D], BF16, tag=f"bl{j}")
                nc.gpsimd.tensor_copy(bl[:], pfl)
                pfb_u.append(bu)
                pfb_l.append(bl)
                if j % 2 == 0:
                    scru = wpool.tile([PL, D], F32, tag="scru")
                    nc.scalar.activation(scru[:], pfu, AF.Square, accum_out=SSQU[:, j:j + 1])
                    scrl = wpool.tile([PH, D], F32, tag="scrl")
                    nc.scalar.activation(scrl[:], pfl, AF.Square, accum_out=SSQL[:, j:j + 1])
                else:
                    scru = wpool.tile([PL, D], F32, tag="scru")
                    nc.vector.scalar_tensor_tensor(scru[:], pfu, 1.0, pfu, ALU.mult, ALU.mult,
                                                   accum_out=SSQU[:, j:j + 1])
                    scrl = wpool.tile([PH, D], F32, tag="scrl")
                    nc.vector.scalar_tensor_tensor(scrl[:], pfl, 1.0, pfl, ALU.mult, ALU.mult,
                                                   accum_out=SSQL[:, j:j + 1])

            # inv patch norms: 1/max(sqrt(ssq), 1e-12)
            INVU = wpool.tile([PL, SPC], F32, tag="invu")
            nc.scalar.sqrt(INVU[:], SSQU[:])
            nc.vector.tensor_scalar(INVU[:], INVU[:], 1e-12, None, op0=ALU.max)
            nc.vector.reciprocal(INVU[:], INVU[:])
            INVL = wpool.tile([PH, SPC], F32, tag="invl")
            nc.scalar.sqrt(INVL[:], SSQL[:])
            nc.vector.tensor_scalar(INVL[:], INVL[:], 1e-12, None, op0=ALU.max)
            nc.vector.reciprocal(INVL[:], INVL[:])

            # --- mask transpose + scale -> zero-padded bf16 weights ---
            # wz flat [p, 1280]: slot stride 160, block j (16 cols) at 160j,
            # matmul window j = flat cols [144j, 144j+128) -> block lands at
            # window offset 16j, all other cols in window are zero.
            mtu_ps = ps_tp.tile([PL, 128], F32, tag="tp")
            nc.tensor.transpose(mtu_ps[:], mask[:, 0:PL], I128[:])
            mtl_ps = ps_tp.tile([PH, 128], F32, tag="tp")
            nc.tensor.transpose(mtl_ps[:], mask[:, PL:P], I128[:])
            wzu = wz_tiles[ci % 2][0]
            wzl = wz_tiles[ci % 2][1]
            wzu3 = wzu[:].rearrange("p (a b) -> p a b", a=SPC)[:, :, 0:K]
            mtu3 = mtu_ps[:].rearrange("p (a b) -> p a b", a=SPC)
            nc.vector.scalar_tensor_tensor(
                wzu3, mtu3, 1.0 / K, INVU[:].unsqueeze(2).broadcast_to([PL, SPC, K]),
                ALU.mult, ALU.mult)
            wzl3 = wzl[:].rearrange("p (a b) -> p a b", a=SPC)[:, :, 0:K]
            mtl3 = mtl_ps[:].rearrange("p (a b) -> p a b", a=SPC)
            nc.vector.scalar_tensor_tensor(
                wzl3, mtl3, 1.0 / K, INVL[:].unsqueeze(2).broadcast_to([PH, SPC, K]),
                ALU.mult, ALU.mult)

            # --- mean-patch matmuls (one accumulation group, M=128) ---
            mp = ps_mpg.tile([128, D], F32, tag="mpg")
            wzuf = wzu[:]
            wzlf = wzl[:]
            for j in range(SPC):
                nc.tensor.matmul(mp[:], wzuf[:, 144 * j:144 * j + 128], pfb_u[j][:],
                                 start=(j == 0), stop=False, skip_group_check=True)
                nc.tensor.matmul(mp[:], wzlf[:, 144 * j:144 * j + 128], pfb_l[j][:],
                                 start=False, stop=(j == SPC - 1), skip_group_check=True)

            # --- cav gather + num/den ---
            csel_ps = ps_big.tile([128, D], F32, tag="big")
            nc.tensor.matmul(csel_ps[:], ohT[:], cav_nr[:], start=True, stop=True)
            csel = wpool.tile([128, D], F32, tag="csels")
            nc.scalar.copy(csel[:], csel_ps[:])
            scrn = wpool.tile([128, D], F32, tag="scrn")
            nc.vector.scalar_tensor_tensor(scrn[:], mp[:], 1.0, csel[:], ALU.mult, ALU.mult,
                                           accum_out=NUM[:, ci:ci + 1])
            scrq = wpool.tile([128, D], F32, tag="scrq")
            nc.scalar.activation(scrq[:], mp[:], AF.Square, accum_out=SSQMP[:, ci:ci + 1])

            # --- dw gather ---
            dw_ps = ps_tp.tile([128, C], F32, tag="tp")
            nc.tensor.matmul(dw_ps[:], LABT[:, 128 * ci:128 * (ci + 1)], wv_n[:], start=True, stop=True)
            scrw = wpool.tile([128, C], F32, tag="scrw")
            nc.vector.scalar_tensor_tensor(scrw[:], dw_ps[:], 1.0, oh[:], ALU.mult, ALU.mult,
                                           accum_out=DW[:, ci:ci + 1])

        # =============== post-loop align-loss reduction ===============
        den = cpool.tile([128, NCH], F32, tag="den")
        nc.scalar.sqrt(den[:], SSQMP[:])
        nc.vector.tensor_scalar(den[:], den[:], 1e-8, None, op0=ALU.max)
        nc.vector.reciprocal(den[:], den[:])
        sim = cpool.tile([128, NCH], F32, tag="sim")
        nc.vector.tensor_tensor(sim[:], NUM[:], den[:], op=ALU.mult)
        pos = cpool.tile([128, NCH], F32, tag="pos")
        nc.vector.tensor_scalar(pos[:], DW[:], 0.1, None, op0=ALU.is_gt)
        t1 = cpool.tile([128, NCH], F32, tag="t1")
        nc.vector.tensor_scalar(t1[:], sim[:], -1.0, 1.0, op0=ALU.mult, op1=ALU.add)  # 1-sim
        nc.vector.tensor_tensor(t1[:], t1[:], DW[:], op=ALU.mult)
        nc.vector.tensor_tensor(t1[:], t1[:], pos[:], op=ALU.mult)
        nc.vector.tensor_reduce(ACCF[:, 1:2], t1[:], axis=AX.X, op=ALU.add)
        nc.vector.tensor_reduce(ACCF[:, 2:3], pos[:], axis=AX.X, op=ALU.add)
        t2 = cpool.tile([128, NCH], F32, tag="t2")
        nc.vector.tensor_scalar(t2[:], sim[:], 1.0, None, op0=ALU.add)  # 1+sim
        t3 = cpool.tile([128, NCH], F32, tag="t3")
        nc.vector.tensor_scalar(t3[:], pos[:], -1.0, 1.0, op0=ALU.mult, op1=ALU.add)  # 1-pos
        nc.vector.tensor_tensor(t2[:], t2[:], t3[:], op=ALU.mult)
        nc.vector.tensor_reduce(ACCF[:, 3:4], t2[:], axis=AX.X, op=ALU.add)

        # =============== partition-sum via ones matmul ===============
        ones = cpool.tile([128, 1], F32, tag="ones")
        nc.gpsimd.memset(ones[:], 1.0)
        fin = ps_mpg.tile([1, 8], F32, tag="mpg")
        nc.tensor.matmul(fin[:], ones[:], ACCF[:], start=True, stop=True)
        fins = cpool.tile([1, 8], F32, tag="fins")
        nc.scalar.copy(fins[:], fin[:])
        nc.sync.dma_start(out_d[:], fins[:])

    return nc


_CACHE = {}


def _get_nc():
    if "nc" not in _CACHE:
        _CACHE["nc"] = _build_module()
    return _CACHE["nc"]


def _make_in_maps(inputs):
    dl = np.ascontiguousarray(inputs["disease_logits"], dtype=np.float32)
    cs = np.ascontiguousarray(inputs["concept_scores"], dtype=np.float32)
    labels = np.asarray(inputs["labels"]).astype(np.int64)
    ps = np.ascontiguousarray(inputs["patch_similarity"], dtype=np.float32)
    pf = np.ascontiguousarray(inputs["patch_features"], dtype=np.float32)
    cav = np.ascontiguousarray(inputs["cav_vectors"], dtype=np.float32)
    wv = np.ascontiguousarray(inputs["w_vote"], dtype=np.float32)
    te = np.ascontiguousarray(inputs["text_embeddings"], dtype=np.float32)
    tki = np.asarray(inputs["top_k_indices"]).astype(np.int64)

    iota80 = np.broadcast_to(np.arange(C, dtype=np.float32), (128, C)).copy()
    ident = np.eye(128, dtype=np.float32)
    ident80 = np.eye(C, dtype=np.float32)

    in_maps = []
    for c in range(N_CORES):
        sl = slice(c * BSH, (c + 1) * BSH)
        lab = labels[sl]
        ohlab = np.zeros((BSH, 7), np.float32)
        ohlab[np.arange(BSH), lab] = 1.0
        confrow = CONF_MAT[lab]
        clsw = CLASS_W[lab][:, None]
        labt = np.zeros((7, BSH * K), np.float32)
        labt[np.repeat(lab, K), np.arange(BSH * K)] = 1.0
        cnt = np.zeros((C, BSH), np.float32)
        np.add.at(cnt, (tki[sl].reshape(-1), np.repeat(np.arange(BSH), K)), 1.0)
        in_maps.append({
            "dl": dl[sl], "cs": cs[sl], "ps": ps[sl], "pf": pf[sl],
            "cav": cav, "wv": wv, "te": te, "cnt": cnt,
            "iota80": iota80, "ident": ident, "ident80": ident80,
            "ohlab": ohlab, "confrow": confrow, "clsw": clsw, "labt": labt,
        })
    return in_maps


def _combine(parts):
    o = np.sum([p["out"].reshape(8) for p in parts], axis=0, dtype=np.float64)
    wce_sum, s_pos, n_pos, s_neg, text_ln, text_diag = o[0], o[1], o[2], o[3], o[4], o[5]
    loss_cls = 0.25 * wce_sum / B
    pos_cnt = max(round(n_pos), 1.0)
    neg_cnt = max(B * K - round(n_pos), 1.0)
    loss_align = s_pos / pos_cnt + 0.1 * s_neg / neg_cnt
    loss_text = (text_ln - text_diag) / (B * K)
    total = loss_cls + ALPHA * loss_align + BETA * loss_text
    return np.float32(total)


def _get_runner():
    """Persistent jitted sharded executable (compiled once per process)."""
    if "runner" in _CACHE:
        return _CACHE["runner"]
    import jax
    from jax.sharding import Mesh, PartitionSpec
    from jax.experimental.shard_map import shard_map
    from concourse import bass2jax

    nc = _get_nc()
    bass2jax.install_neuronx_cc_hook()
    partition_name = nc.partition_id_tensor.name if nc.partition_id_tensor else None
    in_names, out_names, out_avals, zero_outs = [], [], [], []
    for alloc in nc.m.functions[0].allocations:
        if not isinstance(alloc, mybir.MemoryLocationSet):
            continue
        name = alloc.memorylocations[0].name
        if alloc.kind == "ExternalInput":
            if name != partition_name:
                in_names.append(name)
        elif alloc.kind == "ExternalOutput":
            shape = tuple(alloc.tensor_shape)
            dtype = mybir.dt.np(alloc.dtype)
            out_names.append(name)
            out_avals.append(jax.core.ShapedArray(shape, dtype))
            zero_outs.append(np.zeros(shape, dtype))
    n_params = len(in_names)
    all_in = list(in_names) + list(out_names)
    if partition_name is not None:
        all_in.append(partition_name)

    def _body(*args):
        operands = list(args)
        if partition_name is not None:
            operands.append(bass2jax.partition_id_tensor())
        return tuple(bass2jax._bass_exec_p.bind(
            *operands, out_avals=tuple(out_avals), in_names=tuple(all_in),
            out_names=tuple(out_names), lowering_input_output_aliases=(),
            sim_require_finite=True, sim_require_nnan=True, nc=nc))

    devices = jax.devices()[:N_CORES]
    mesh = Mesh(np.asarray(devices), ("core",))
    fn = jax.jit(shard_map(_body, mesh=mesh,
                           in_specs=(PartitionSpec("core"),) * (n_params + len(out_names)),
                           out_specs=(PartitionSpec("core"),) * len(out_names),
                           check_rep=False), keep_unused=True)

    def run(in_maps):
        concat_in = [np.concatenate([in_maps[c][nm] for c in range(N_CORES)], axis=0)
                     for nm in in_names]
        concat_zeros = [np.zeros((N_CORES * z.shape[0], *z.shape[1:]), z.dtype)
                        for z in zero_outs]
        outs = fn(*concat_in, *concat_zeros)
        arr = np.asarray(outs[0]).reshape(N_CORES, zero_outs[0].shape[0], -1)
        return [{"out": arr[c]} for c in range(N_CORES)]

    _CACHE["runner"] = run
    return run


def kernel(**inputs) -> np.ndarray:
    run = _get_runner()
    in_maps = _make_in_maps(inputs)
    return _combine(run(in_maps))

